# revision 3
# baseline (speedup 1.0000x reference)
"""LM-Infinite sparse attention kernel for Trainium2 (8 NeuronCores).

Reference semantics: causal attention with additive bias min(j-i, 2048) on
logits, masked to keys j in [0, n_global) U [i-2047, i].  Because the bias
decays as e^(j-i), any key at distance > ~90 underflows to exactly 0 in f32
(global sink keys are only reachable outside the local window at distance
>= 1949, where e^-1949 == 0.0f), so the f32 output equals a sliding-window
attention with a ~128..256 key window.  We compute, per 128-query tile, the
previous and diagonal 128-key blocks: every query sees >= 129 most recent
keys; dropped keys have weight < e^-125 relative.

Softmax is computed without the row-max subtraction (logits <= |qk|/sqrt(D)
~ +-8, exp never overflows): P = exp(qk*scale) * Bias, with Bias = e^(j-i)
(0 where masked) precomputed on host as two 128x128 tiles.  The denominator
is fused into the PV matmul by appending a ones-column to V.  Everything is
computed in the transposed space S^T[j, q] so that P^T is directly the lhsT
of the PV matmul and V needs no transpose.

Sharding: core = b*4 + cc handles batch b, queries [cc*2048, (cc+1)*2048).
K/V are passed with a 128-key halo; core cc=0 gets a zeroed halo whose
bias tile is all-zero (masked multiplicatively).
"""

import math
import numpy as np

import concourse.bass as bass
import concourse.mybir as mybir
import concourse.tile as tile
from concourse import bacc
from concourse.masks import make_identity
from concourse.bass_utils import run_bass_kernel_spmd

B, S, D = 2, 8192, 128
NCORES = 8
CHUNK = S // 4          # 2048 queries per core
NQT = CHUNK // 128      # 16 query tiles per core
KLEN = CHUNK + 128      # key slice incl. halo
F32 = mybir.dt.float32
SCALE = 1.0 / math.sqrt(D)

_CACHE = {}


def _build_bass():
    nc = bacc.Bacc("TRN2", target_bir_lowering=False, debug=False)
    q = nc.dram_tensor("q", [CHUNK, D], F32, kind="ExternalInput").ap()
    k = nc.dram_tensor("k", [KLEN, D], F32, kind="ExternalInput").ap()
    v = nc.dram_tensor("v", [KLEN, D], F32, kind="ExternalInput").ap()
    bias = nc.dram_tensor("bias", [128, 384], F32, kind="ExternalInput").ap()
    out = nc.dram_tensor("out", [CHUNK, D], F32, kind="ExternalOutput").ap()

    with tile.TileContext(nc) as tc:
        with (
            tc.tile_pool(name="const", bufs=1) as const,
            tc.tile_pool(name="qtbuf", bufs=1) as qtbuf,
            tc.tile_pool(name="loads", bufs=4) as loads,
            tc.tile_pool(name="kts", bufs=3) as kts,
            tc.tile_pool(name="vs", bufs=4) as vsp,
            tc.tile_pool(name="ps", bufs=4) as psp,
            tc.tile_pool(name="outs", bufs=4) as outs,
            tc.tile_pool(name="tpsum", bufs=2, space="PSUM") as tpsum,
            tc.tile_pool(name="spsum", bufs=2, space="PSUM") as spsum,
            tc.tile_pool(name="opsum", bufs=3, space="PSUM") as opsum,
        ):
            ident = const.tile([128, 128], F32)
            make_identity(nc, ident[:])
            bt = const.tile([128, 384], F32)
            nc.sync.dma_start(bt[:], bias[:])

            # Q^T staging buffer: [d, q] for the whole 2048-query chunk.
            QT = qtbuf.tile([128, CHUNK], F32)
            for qt in range(NQT):
                qa = loads.tile([128, 128], F32, tag="qa")
                nc.sync.dma_start(qa[:], q[qt * 128:(qt + 1) * 128, :])
                ps = tpsum.tile([128, 128], F32, tag="tp")
                nc.tensor.transpose(ps[:], qa[:], ident[:])
                nc.vector.tensor_copy(QT[:, qt * 128:(qt + 1) * 128], ps[:])

            o_acc = {}
            for kb in range(-1, NQT):
                # K block -> K^T (lhsT for the S^T matmul)
                ka = loads.tile([128, 128], F32, tag="ka")
                nc.sync.dma_start(ka[:], k[(kb + 1) * 128:(kb + 2) * 128, :])
                ps = tpsum.tile([128, 128], F32, tag="tp")
                nc.tensor.transpose(ps[:], ka[:], ident[:])
                kt = kts.tile([128, 128], F32, tag="kt")
                nc.vector.tensor_copy(kt[:], ps[:])

                # rhs: Q^T columns of the query tiles that see this key block.
                # Layout: [diag-half (qt==kb) | prev-half (qt==kb+1)].
                if kb == -1:
                    q0, n, b0 = 0, 128, 256          # prev-only, PREVZERO bias
                elif kb == NQT - 1:
                    q0, n, b0 = kb * 128, 128, 0     # diag-only, DIAG bias
                else:
                    q0, n, b0 = kb * 128, 256, 0     # [DIAG | PREV] bias
                st = spsum.tile([128, n], F32, tag="st")
                nc.tensor.matmul(st[:], kt[:], QT[:, q0:q0 + n],
                                 start=True, stop=True)

                # P^T = exp(S^T * scale) .* e^(j-i)  (0 where masked)
                p0 = psp.tile([128, n], F32, tag="p0")
                nc.scalar.activation(p0[:], st[:],
                                     mybir.ActivationFunctionType.Exp,
                                     scale=SCALE)
                p = psp.tile([128, n], F32, tag="p")
                nc.vector.tensor_mul(p[:], p0[:], bt[:, b0:b0 + n])

                # V block with a ones-column (fused softmax denominator).
                vt = vsp.tile([128, 129], F32, tag="vt")
                nc.sync.dma_start(vt[:, 0:128], v[(kb + 1) * 128:(kb + 2) * 128, :])
                nc.gpsimd.memset(vt[:, 128:129], 1.0)

                # PV: prev-half opens O[kb+1]; diag-half closes O[kb].
                if kb + 1 <= NQT - 1:
                    ot = opsum.tile([128, 129], F32, tag="ot")
                    o_acc[kb + 1] = ot
                    nc.tensor.matmul(ot[:], p[:, n - 128:n], vt[:],
                                     start=True, stop=False)
                if kb >= 0:
                    ot = o_acc.pop(kb)
                    nc.tensor.matmul(ot[:], p[:, 0:128], vt[:],
                                     start=False, stop=True)
                    rec = outs.tile([128, 1], F32, tag="rec")
                    nc.vector.reciprocal(rec[:], ot[:, 128:129])
                    ob = outs.tile([128, 128], F32, tag="ob")
                    nc.vector.tensor_scalar_mul(ob[:], ot[:, 0:128], rec[:])
                    nc.sync.dma_start(out[kb * 128:(kb + 1) * 128, :], ob[:])

    nc.compile()
    return nc


def _bias_tiles(is_first_chunk: bool) -> np.ndarray:
    jj = np.arange(128, dtype=np.float64)[:, None]
    uu = np.arange(128, dtype=np.float64)[None, :]
    diag = np.where(jj <= uu, np.exp(jj - uu), 0.0).astype(np.float32)
    prev = np.exp(jj - 128 - uu).astype(np.float32)
    prevzero = np.zeros_like(prev) if is_first_chunk else prev
    return np.concatenate([diag, prev, prevzero], axis=1)  # [128, 384]


def kernel(q: np.ndarray, k: np.ndarray, v: np.ndarray) -> np.ndarray:
    return _run(q, k, v)[0]


def _run(q, k, v, trace=False, tmpdir=None):
    q = np.ascontiguousarray(np.asarray(q, dtype=np.float32))
    k = np.ascontiguousarray(np.asarray(k, dtype=np.float32))
    v = np.ascontiguousarray(np.asarray(v, dtype=np.float32))

    if "nc" not in _CACHE:
        _CACHE["nc"] = _build_bass()
    nc = _CACHE["nc"]

    in_maps = []
    for core in range(NCORES):
        b, cc = divmod(core, 4)
        lo, hi = cc * CHUNK, (cc + 1) * CHUNK
        if cc == 0:
            pad = np.zeros((128, D), dtype=np.float32)
            ks = np.concatenate([pad, k[b, lo:hi]], axis=0)
            vs = np.concatenate([pad, v[b, lo:hi]], axis=0)
        else:
            ks = k[b, lo - 128:hi]
            vs = v[b, lo - 128:hi]
        in_maps.append({
            "q": np.ascontiguousarray(q[b, lo:hi]),
            "k": np.ascontiguousarray(ks),
            "v": np.ascontiguousarray(vs),
            "bias": _bias_tiles(cc == 0),
        })

    res = run_bass_kernel_spmd(nc, in_maps, list(range(NCORES)),
                               trace=trace, tmpdir=tmpdir)
    out = np.empty((B, S, D), dtype=np.float32)
    for core in range(NCORES):
        b, cc = divmod(core, 4)
        out[b, cc * CHUNK:(cc + 1) * CHUNK] = res.results[core]["out"]
    return out, res


# revision 7
# speedup vs baseline: 1.4783x; 1.4783x over previous
"""LM-Infinite sparse attention kernel for Trainium2 (8 NeuronCores).

Reference semantics: causal attention with additive bias min(j-i, 2048) on
logits, masked to keys j in [0, n_global) U [i-2047, i].  Because the bias
decays as e^(j-i), any key at distance > ~90 underflows to exactly 0 in f32
(global sink keys are only reachable outside the local window at distance
>= 1949, where e^-1949 == 0.0f), so the f32 output equals a sliding-window
attention with a ~128..256 key window.  We compute, per 128-query tile, the
previous and diagonal 128-key blocks: every query sees >= 129 most recent
keys; dropped keys have weight < e^-125 relative.

Softmax is computed without the row-max subtraction (logits <= |qk|/sqrt(D)
~ +-8, exp never overflows): P = exp(qk*scale) * Bias, with Bias = e^(j-i)
(0 where masked) precomputed on host as two 128x128 tiles.  The denominator
is fused into the PV matmul by appending a ones-column to V.  Everything is
computed in the transposed space S^T[j, q] so that P^T is directly the lhsT
of the PV matmul and V needs no transpose.

Sharding: core = b*4 + cc handles batch b, queries [cc*2048, (cc+1)*2048).
K/V are passed with a 128-key halo; core cc=0 gets a zeroed halo whose
bias tile is all-zero (masked multiplicatively).
"""

import math
import numpy as np

import concourse.bass as bass
import concourse.mybir as mybir
import concourse.tile as tile
from concourse import bacc
from concourse.masks import make_identity
from concourse.bass_utils import run_bass_kernel_spmd

B, S, D = 2, 8192, 128
NCORES = 8
CHUNK = S // 4          # 2048 queries per core
NQT = CHUNK // 128      # 16 query tiles per core
NKB = NQT + 1           # 17 key blocks incl. halo
KLEN = CHUNK + 128      # key slice incl. halo
F32 = mybir.dt.float32
F32R = mybir.dt.float32r
SCALE = 1.0 / math.sqrt(D)
VW = 129                # V block width incl. ones-column

_CACHE = {}


def _build_bass(use_f32r=True):
    # float32r (reduced-precision full-rate PE mode) operands must be
    # written by a rounding compute instruction, not raw DMA: KT/QT/P are
    # rounded by their producing DVE copies; V gets one extra rounding
    # copy (VN -> VNR).  Transposes stay f32 (their inputs come from DMA).
    rdt = F32R if use_f32r else F32
    nc = bacc.Bacc("TRN2", target_bir_lowering=False, debug=False)
    q = nc.dram_tensor("q", [CHUNK, D], F32, kind="ExternalInput").ap()
    k = nc.dram_tensor("k", [KLEN, D], F32, kind="ExternalInput").ap()
    v = nc.dram_tensor("v", [KLEN, D], F32, kind="ExternalInput").ap()
    bias = nc.dram_tensor("bias", [128, 384], F32, kind="ExternalInput").ap()
    out = nc.dram_tensor("out", [CHUNK, D], F32, kind="ExternalOutput").ap()

    with tile.TileContext(nc) as tc:
        with (
            tc.tile_pool(name="const", bufs=1) as const,
            tc.tile_pool(name="big", bufs=1) as big,
            tc.tile_pool(name="kts", bufs=4) as kts,
            tc.tile_pool(name="ps", bufs=4) as psp,
            tc.tile_pool(name="outs", bufs=4) as outs,
            tc.tile_pool(name="tpsum", bufs=2, space="PSUM") as tpsum,
            tc.tile_pool(name="spsum", bufs=2, space="PSUM") as spsum,
            tc.tile_pool(name="opsum", bufs=3, space="PSUM") as opsum,
        ):
            ident = const.tile([128, 128], F32)
            make_identity(nc, ident[:])
            bt = const.tile([128, 384], F32)
            nc.sync.dma_start(bt[:], bias[:])

            # Bulk loads: a few large strided DMAs instead of per-tile ones.
            # QN/KN hold 128x128 blocks side by side; VN holds 129-wide
            # blocks (V | ones-column), padded so every PV rhs window can be
            # 256 wide (float32r needs free dim >= 256 for full rate).
            QN = big.tile([128, CHUNK], F32)
            KN = big.tile([128, NKB * 128], F32)
            VN = big.tile([128, NKB * VW + 127], F32)
            q_src = q.rearrange("(n p) d -> p n d", p=128)
            qn3 = QN[:].rearrange("p (n d) -> p n d", d=128)
            nc.sync.dma_start(qn3[:, 0:8, :], q_src[:, 0:8, :])
            nc.sync.dma_start(qn3[:, 8:16, :], q_src[:, 8:16, :])
            k_src = k.rearrange("(n p) d -> p n d", p=128)
            kn3 = KN[:].rearrange("p (n d) -> p n d", d=128)
            nc.sync.dma_start(kn3[:, 0:9, :], k_src[:, 0:9, :])
            nc.sync.dma_start(kn3[:, 9:NKB, :], k_src[:, 9:NKB, :])
            v_src = v.rearrange("(n p) d -> p n d", p=128)
            vn3 = VN[:, 0:NKB * VW].rearrange("p (n e) -> p n e", e=VW)
            nc.sync.dma_start(vn3[:, 0:9, 0:128], v_src[:, 0:9, :])
            nc.sync.dma_start(vn3[:, 9:NKB, 0:128], v_src[:, 9:NKB, :])
            nc.gpsimd.memset(vn3[:, :, 128:129], 1.0)
            nc.gpsimd.memset(VN[:, NKB * VW:], 0.0)
            # rounding pass for the PV rhs
            VNR = big.tile([128, NKB * VW + 127], rdt)
            nc.vector.tensor_copy(VNR[:], VN[:])

            # Q^T staging: [d, q] for the whole chunk via PE transposes.
            QT = big.tile([128, CHUNK], rdt)
            for qt in range(NQT):
                ps = tpsum.tile([128, 128], F32, tag="tp")
                nc.tensor.matmul(ps[:], QN[:, qt * 128:(qt + 1) * 128],
                                 ident[:], is_transpose=True)
                nc.vector.tensor_copy(QT[:, qt * 128:(qt + 1) * 128], ps[:])

            OB = big.tile([128, CHUNK], F32)

            o_acc = {}
            for kb in range(-1, NQT):
                kb2 = kb + 1
                ps = tpsum.tile([128, 128], F32, tag="tp")
                nc.tensor.matmul(ps[:], KN[:, kb2 * 128:(kb2 + 1) * 128],
                                 ident[:], is_transpose=True)
                kt = kts.tile([128, 128], rdt, tag="kt")
                nc.vector.tensor_copy(kt[:], ps[:])

                # rhs: Q^T columns of the query tiles that see this block:
                # [diag-half (qt==kb) | prev-half (qt==kb+1)].
                if kb == -1:
                    q0, n, b0 = 0, 128, 256          # prev-only, PREVZERO bias
                elif kb == NQT - 1:
                    q0, n, b0 = kb * 128, 128, 0     # diag-only, DIAG bias
                else:
                    q0, n, b0 = kb * 128, 256, 0     # [DIAG | PREV] bias
                st = spsum.tile([128, n], F32, tag="st")
                nc.tensor.matmul(st[:], kt[:], QT[:, q0:q0 + n],
                                 start=True, stop=True)

                # P^T = exp(S^T * scale) .* e^(j-i)  (0 where masked)
                p0 = psp.tile([128, n], F32, tag="p0")
                nc.scalar.activation(p0[:], st[:],
                                     mybir.ActivationFunctionType.Exp,
                                     scale=SCALE)
                p = psp.tile([128, n], rdt, tag="p")
                nc.vector.tensor_mul(p[:], p0[:], bt[:, b0:b0 + n])

                # PV (fused denominator): rhs is the 256-wide window
                # [V_kb | ones | junk]; prev-half opens O[kb+1], diag-half
                # closes O[kb].  Junk columns >= 129 are never read.
                vwin = VNR[:, kb2 * VW:kb2 * VW + 256]
                if kb + 1 <= NQT - 1:
                    ot = opsum.tile([128, 256], F32, tag="ot")
                    o_acc[kb + 1] = ot
                    nc.tensor.matmul(ot[:], p[:, n - 128:n], vwin,
                                     start=True, stop=False)
                if kb >= 0:
                    ot = o_acc.pop(kb)
                    nc.tensor.matmul(ot[:], p[:, 0:128], vwin,
                                     start=False, stop=True)
                    rec = outs.tile([128, 1], F32, tag="rec")
                    nc.vector.reciprocal(rec[:], ot[:, 128:129])
                    nc.vector.tensor_scalar_mul(
                        OB[:, kb * 128:(kb + 1) * 128], ot[:, 0:128], rec[:])

            ob3 = OB[:].rearrange("p (n d) -> p n d", d=128)
            o_dst = out.rearrange("(n p) d -> p n d", p=128)
            nc.sync.dma_start(o_dst[:, 0:8, :], ob3[:, 0:8, :])
            nc.sync.dma_start(o_dst[:, 8:16, :], ob3[:, 8:16, :])

    nc.compile()
    return nc


def _bias_tiles(is_first_chunk: bool) -> np.ndarray:
    jj = np.arange(128, dtype=np.float64)[:, None]
    uu = np.arange(128, dtype=np.float64)[None, :]
    diag = np.where(jj <= uu, np.exp(jj - uu), 0.0).astype(np.float32)
    prev = np.exp(jj - 128 - uu).astype(np.float32)
    prevzero = np.zeros_like(prev) if is_first_chunk else prev
    return np.concatenate([diag, prev, prevzero], axis=1)  # [128, 384]


def kernel(q: np.ndarray, k: np.ndarray, v: np.ndarray) -> np.ndarray:
    return _run(q, k, v)[0]


def _run(q, k, v, trace=False, tmpdir=None, use_f32r=True):
    q = np.ascontiguousarray(np.asarray(q, dtype=np.float32))
    k = np.ascontiguousarray(np.asarray(k, dtype=np.float32))
    v = np.ascontiguousarray(np.asarray(v, dtype=np.float32))

    key = ("nc", use_f32r)
    if key not in _CACHE:
        _CACHE[key] = _build_bass(use_f32r)
    nc = _CACHE[key]

    in_maps = []
    for core in range(NCORES):
        b, cc = divmod(core, 4)
        lo, hi = cc * CHUNK, (cc + 1) * CHUNK
        if cc == 0:
            pad = np.zeros((128, D), dtype=np.float32)
            ks = np.concatenate([pad, k[b, lo:hi]], axis=0)
            vs = np.concatenate([pad, v[b, lo:hi]], axis=0)
        else:
            ks = k[b, lo - 128:hi]
            vs = v[b, lo - 128:hi]
        in_maps.append({
            "q": np.ascontiguousarray(q[b, lo:hi]),
            "k": np.ascontiguousarray(ks),
            "v": np.ascontiguousarray(vs),
            "bias": _bias_tiles(cc == 0),
        })

    res = run_bass_kernel_spmd(nc, in_maps, list(range(NCORES)),
                               trace=trace, tmpdir=tmpdir)
    out = np.empty((B, S, D), dtype=np.float32)
    for core in range(NCORES):
        b, cc = divmod(core, 4)
        out[b, cc * CHUNK:(cc + 1) * CHUNK] = res.results[core]["out"]
    return out, res
